# revision 28
# baseline (speedup 1.0000x reference)
"""Additive (Bahdanau) attention on 8 TRN2 NeuronCores.

scores[b,i,j] = sum_h wv_h * tanh(qp[b,i,h] + kp[b,j,h]),  qp = q@Wq.T, kp = k@Wk.T
masked softmax over j, then attn @ values.

Math: tanh(s) ~ c0*s + sum_n b_n sin(n w s) over two frequency ladders
L0=(w0,(1,2,4)) and L1=(w1,(4,8)); sin(w(q+k)) = sin(wq)cos(wk)+cos(wq)sin(wk)
turns the (B,NQ,NK,H) tanh contraction into TensorEngine matmuls over Fourier
features. ACT Sin is only accurate for |arg|<=3.15 so cosines come from
Sin(-w|x| + pi/2) and w0 is capped at 0.46; higher harmonics via double-angle
ladders (sp_n = sin(nwx)/n raw, interior cosines exactified, leaf harmonics
use ct_n = cos(n/2 wx)^2 with rank-1 beta corrections). Per-query-constant
score terms are dropped: softmax is row-invariant and the host divides by z.

Device-side structure:
- host pre-packs ALL inputs (transposed, bf16) in two large DMAs;
- q and k projections land in ONE psum tile laid out (hc, [q|k]) so every
  ladder op covers q-side and k-side of both h-chunks in one instruction;
- the ladder chain runs entirely on the DVE (gpsimd streaming poisons the
  shared SBUF port); the wv*coef folds run on ScalarE (Copy + AP scale) for
  the early harmonics and on DVE after the chain for the late ones;
- valid_lens mask is folded into zeroed value rows + an appended ones-column
  (z = sum(E) falls out of the AV matmul); exp runs without max-subtraction;
- dummy matmuls on a memset tile + feature-dependent fillers keep the PE's
  HAM clock gate warm so the main matmuls run at 2.4 GHz.

Sharding: keys are sharded across cores at 64-key granularity. Each core gets
(batch b, key-range) with a common per-core KPAD = 64*L chosen so the
ceil(vl_b/64) units of all batches bin-pack into 8 single-batch bins; every
core computes partial ov[b] = E@V and z[b] = sum(E) over its key range for ALL
128 queries of its batch, and the host combines: out = sum(ov) / sum(z).
"""
import sys
import numpy as np

try:
    import concourse.bass as bass
except ImportError:
    sys.path.insert(0, "/opt/trn_rl_repo")
    import concourse.bass as bass
import concourse.bacc as bacc
import concourse.mybir as mybir
from contextlib import ExitStack
from concourse.tile import TileContext
from concourse.bass_utils import run_bass_kernel_spmd

F32 = mybir.dt.float32
BF = mybir.dt.bfloat16
AF = mybir.ActivationFunctionType
ALU = mybir.AluOpType

B, NQ, NK, H, DV = 4, 128, 1024, 256, 256
PIHALF = float(np.pi / 2)

# tanh(x) ~ C0*x + sum b_(li,n) sin(n * w_li * x); weighted LSQ fit over N(0,sigma^2)
CFG = ((0.46, (1, 2, 4)), (0.34, (4, 8)))
SIGMA = 1.665


def _fit():
    xs = np.linspace(-6 * SIGMA, 6 * SIGMA, 8001)
    wts = np.exp(-xs ** 2 / (2 * SIGMA ** 2))
    cols = [xs] + [np.sin(n * w * xs) for (w, hs) in CFG for n in hs]
    A = np.stack(cols, 1)
    Wm = np.sqrt(wts)[:, None]
    coef, *_ = np.linalg.lstsq(A * Wm, np.tanh(xs) * Wm[:, 0], rcond=None)
    c0 = float(coef[0])
    bs = {}
    i = 1
    for li, (w, hs) in enumerate(CFG):
        for n in hs:
            bs[(li, n)] = float(coef[i]); i += 1
    return c0, bs


C0, BS = _fit()
W0, W1 = CFG[0][0], CFG[1][0]

# af coefficient per pair-tile: interior n -> n*b_n ; leaf n -> 2n*b_n.
# AF_ORDER is the feature-readiness order used for the af ops and wvq cols.
AF_ORDER = ("p1_0", "p4_1", "p8_1", "p2_0", "p4_0")
AF_COEF = {
    "p1_0": BS[(0, 1)],
    "p2_0": 2.0 * BS[(0, 2)],
    "p4_0": 8.0 * BS[(0, 4)],      # L0 leaf (n=4)
    "p4_1": 4.0 * BS[(1, 4)],      # L1 interior
    "p8_1": 16.0 * BS[(1, 8)],     # L1 leaf (n=8)
}


def build_program(KPAD):
    KC = (KPAD + 127) // 128
    M = 128 + KPAD                  # per-hc ladder width (q part | k part)
    S = ((M + 511) // 512) * 512    # bank-aligned hc stride in the prj psum tile
    W = 2 * M                       # full ladder width (both h-chunks)
    # input 1a (sync, first): wkT | kT — gates kprj -> the sin backbone;
    # input 1b: qT | wqT;  input 2: ident | ukb | vv
    N1A = 512 + 2 * KPAD
    N1B = 256 + 512
    N2 = 128 + 768 + KC * 257

    nc = bacc.Bacc("TRN2", target_bir_lowering=False, debug=False, num_devices=8)
    d_in1a = nc.declare_dram_parameter("in1a", [128, N1A], BF, isOutput=False)
    d_in1b = nc.declare_dram_parameter("in1b", [128, N1B], BF, isOutput=False)
    d_in2 = nc.declare_dram_parameter("in2", [128, N2], BF, isOutput=False)
    d_wvq = nc.declare_dram_parameter("wvq", [128, 2 * len(AF_ORDER)], F32,
                                      isOutput=False)
    d_o = nc.declare_dram_parameter("o", [NQ, 257], F32, isOutput=True)

    with TileContext(nc) as tc, ExitStack() as ex:
        cpool = ex.enter_context(tc.tile_pool(name="consts", bufs=1))
        fpool = ex.enter_context(tc.tile_pool(name="feat", bufs=1))
        wpool = ex.enter_context(tc.tile_pool(name="work", bufs=1))
        pprj = ex.enter_context(tc.tile_pool(name="pprj", bufs=1, space="PSUM"))
        psc = ex.enter_context(tc.tile_pool(name="psc", bufs=1, space="PSUM"))
        pov = ex.enter_context(tc.tile_pool(name="pov", bufs=1, space="PSUM"))
        ptp = ex.enter_context(
            tc.tile_pool(name="ptp", bufs=(1 if S > 512 else 2), space="PSUM"))
        pwm = ex.enter_context(tc.tile_pool(name="pwm", bufs=1, space="PSUM"))

        # ---------------- DMAs (one ring, ordered by need) ----------------
        in1a = cpool.tile([128, N1A], BF, name="in1a", tag="in1a")
        nc.sync.dma_start(in1a[:], d_in1a[:])
        in1b = cpool.tile([128, N1B], BF, name="in1b", tag="in1b")
        nc.sync.dma_start(in1b[:], d_in1b[:])
        in2 = cpool.tile([128, N2], BF, name="in2", tag="in2")
        nc.sync.dma_start(in2[:], d_in2[:])
        wvq = cpool.tile([128, 2 * len(AF_ORDER)], F32, name="wvq", tag="wvq")
        nc.sync.dma_start(wvq[:], d_wvq[:])
        wkT = in1a[:, 0:512]
        kT = in1a[:, 512:512 + 2 * KPAD]
        qT = in1b[:, 0:256]
        wqT = in1b[:, 256:768]
        ident = in2[:, 0:128]
        ukb = in2[:, 128:128 + 768]
        vv = in2[:, 896:896 + KC * 257]

        pihalf = cpool.tile([128, 1], F32, name="pihalf", tag="pihalf")
        nc.vector.memset(pihalf[:], PIHALF)
        # junk tile: lets PE warmup matmuls start before any DMA lands
        wj = cpool.tile([128, 384], BF, name="wj", tag="wj")
        nc.vector.memset(wj[:], 1.0)

        # PE warmup into a scratch psum bank: the initial N=384 burst spans
        # >3.4us so the HAM SHORT window actually fires and unthrottles the PE
        wps = pwm.tile([128, 512], F32, name="wps", tag="wps")
        wcnt = [0]

        def warm(k, rhs=None, n=384):
            for _ in range(k):
                nc.tensor.matmul(wps[:, 0:n], wj[:, 0:128],
                                 wj[:] if rhs is None else rhs,
                                 start=(wcnt[0] == 0), stop=False,
                                 skip_group_check=True)
                wcnt[0] += 1

        warm(5)

        # ---------------- projections into one psum tile ----------------
        # prj cols: hc*S + [0:128 q | 128:128+KPAD k]; k first (its DMA lands
        # first and it gates the sin backbone). k split at psum bank
        # boundaries when M > 512.
        prj = pprj.tile([128, 2 * S], F32, name="prj", tag="prj")
        kpieces = []
        a0 = 128
        while a0 < M:
            a1 = min(((a0 // 512) + 1) * 512, M)
            kpieces.append((a0, a1))
            a0 = a1
        for hc in range(2):
            for (a0, a1) in kpieces:
                for dc in range(2):
                    nc.tensor.matmul(prj[:, hc * S + a0: hc * S + a1],
                                     wkT[:, dc * 256 + hc * 128: dc * 256 + (hc + 1) * 128],
                                     kT[:, dc * KPAD + (a0 - 128): dc * KPAD + (a1 - 128)],
                                     start=(dc == 0), stop=(dc == 1))
        for hc in range(2):
            for dc in range(2):
                nc.tensor.matmul(prj[:, hc * S: hc * S + 128],
                                 wqT[:, dc * 256 + hc * 128: dc * 256 + (hc + 1) * 128],
                                 qT[:, dc * NQ:(dc + 1) * NQ],
                                 start=(dc == 0), stop=(dc == 1))

        prjV = prj[:].rearrange("p (a j) -> p a j", a=2)[:, :, 0:M]

        def v3(tile_slice):
            return tile_slice.rearrange("p (a j) -> p a j", a=2)

        # ---------------- feature tiles ----------------
        # pair tiles [128, 2*W]: cols = f*W + hc*M + [0:128 q | 128:M k]
        p1_0 = fpool.tile([128, 2 * W], BF, name="p1_0", tag="p1_0")
        p2_0 = fpool.tile([128, 2 * W], BF, name="p2_0", tag="p2_0")
        p4_0 = fpool.tile([128, 2 * W], BF, name="p4_0", tag="p4_0")
        p4_1 = fpool.tile([128, 2 * W], BF, name="p4_1", tag="p4_1")
        p8_1 = fpool.tile([128, 2 * W], BF, name="p8_1", tag="p8_1")
        PT = {"p1_0": p1_0, "p2_0": p2_0, "p4_0": p4_0, "p4_1": p4_1, "p8_1": p8_1}
        absx = fpool.tile([128, W], F32, name="absx", tag="absx")
        s1b = fpool.tile([128, W], BF, name="s1b", tag="s1b")
        c1b = fpool.tile([128, W], BF, name="c1b", tag="c1b")
        sp2b = fpool.tile([128, W], BF, name="sp2b", tag="sp2b")
        ct2b = fpool.tile([128, W], BF, name="ct2b", tag="ct2b")
        c2b = fpool.tile([128, W], BF, name="c2b", tag="c2b")
        ct4b = fpool.tile([128, W], BF, name="ct4b", tag="ct4b")
        ct2a = fpool.tile([128, W], BF, name="ct2a", tag="ct2a")

        # ---------------- beta-linear mains (only need kT) ----------------
        sc_ps = psc.tile([NQ, KPAD], F32, name="sc", tag="sc")
        nmain = 2 + 4 * 5 + 4
        mi = [0]

        def main(lhsT, rhs):
            nc.tensor.matmul(sc_ps[:, :], lhsT, rhs,
                             start=(mi[0] == 0), stop=(mi[0] == nmain - 1))
            mi[0] += 1

        for dc in range(2):
            main(ukb[:, dc * 128:(dc + 1) * 128], kT[:, dc * KPAD:(dc + 1) * KPAD])

        # ---------------- ladder heads (ACT); L1 first for chain latency ----------------
        nc.scalar.activation(v3(s1b[:]), prjV, AF.Sin, scale=float(W1))
        nc.scalar.activation(v3(absx[:]), prjV, AF.Abs)
        nc.scalar.activation(c1b[:], absx[:], AF.Sin, scale=float(-W1),
                             bias=pihalf[:, 0:1])
        nc.scalar.activation(v3(p1_0[:, 0:W]), prjV, AF.Sin, scale=float(W0))
        nc.scalar.activation(p1_0[:, W:2 * W], absx[:], AF.Sin, scale=float(-W0),
                             bias=pihalf[:, 0:1])

        # HAM fillers: junk matmuls gated on ladder outputs keep the PE's
        # activity window covered while it waits for the main matmul inputs
        warm(2, rhs=s1b[:, 0:384])
        warm(2, rhs=c1b[:, 0:384])

        # af tiles: wv*coef fold on the q-side features. Early harmonics run
        # on ScalarE (Copy with per-partition AP scale) in the post-sin
        # shadow; late harmonics on DVE woven into the chain.
        afs = {name: fpool.tile([128, 512], BF, name=f"af{name}", tag=f"af{name}")
               for name in AF_ORDER}

        def af_op(name, eng):
            ni = AF_ORDER.index(name)
            t = afs[name]
            src3 = PT[name][:].rearrange("p (f x) -> p f x", f=2)
            for hc in range(2):
                out_ap = t[:, hc * 256:(hc + 1) * 256].rearrange(
                    "p (f q) -> p f q", f=2)
                src_ap = src3[:, :, hc * M: hc * M + 128]
                sc1 = wvq[:, 2 * ni + hc: 2 * ni + hc + 1]
                if eng == "v":
                    nc.vector.tensor_scalar(out_ap, src_ap, sc1, None, ALU.mult)
                else:
                    nc.scalar.mul(out_ap, src_ap, sc1)

        # ---------------- chains (all DVE; af weaved at feature readiness) ----------------
        # L1: sp2b=s1b*c1b, ct2b=c1b^2, c2b=2ct2b-1, sp4_1=sp2b*c2b,
        #     ct4b=c2b^2, c4_1=2ct4b-1, sp8_1=sp4_1*c4_1, ct8_1=c4_1^2
        nc.vector.tensor_tensor(sp2b[:], s1b[:], c1b[:], ALU.mult)
        nc.vector.tensor_tensor(ct2b[:], c1b[:], c1b[:], ALU.mult)
        nc.vector.tensor_scalar(c2b[:], ct2b[:], 2.0, -1.0, ALU.mult, ALU.add)
        nc.vector.tensor_tensor(p4_1[:, 0:W], sp2b[:], c2b[:], ALU.mult)
        nc.vector.tensor_tensor(ct4b[:], c2b[:], c2b[:], ALU.mult)
        nc.vector.tensor_scalar(p4_1[:, W:2 * W], ct4b[:], 2.0, -1.0, ALU.mult, ALU.add)
        nc.vector.tensor_tensor(p8_1[:, 0:W], p4_1[:, 0:W], p4_1[:, W:2 * W], ALU.mult)
        nc.vector.tensor_tensor(p8_1[:, W:2 * W], p4_1[:, W:2 * W], p4_1[:, W:2 * W],
                                ALU.mult)
        af_op("p8_1", "v")
        # L0: sp2_0=s1_0*c1_0 -> p2_0 f0, ct2a=c1_0^2, c2_0=2ct2a-1 -> p2_0 f1,
        #     sp4_0=sp2_0*c2_0 -> p4_0 f0, ct4_0=c2_0^2 -> p4_0 f1
        nc.vector.tensor_tensor(p2_0[:, 0:W], p1_0[:, 0:W], p1_0[:, W:2 * W], ALU.mult)
        nc.vector.tensor_tensor(ct2a[:], p1_0[:, W:2 * W], p1_0[:, W:2 * W], ALU.mult)
        nc.vector.tensor_scalar(p2_0[:, W:2 * W], ct2a[:], 2.0, -1.0, ALU.mult, ALU.add)
        af_op("p2_0", "v")
        nc.vector.tensor_tensor(p4_0[:, 0:W], p2_0[:, 0:W], p2_0[:, W:2 * W], ALU.mult)
        nc.vector.tensor_tensor(p4_0[:, W:2 * W], p2_0[:, W:2 * W], p2_0[:, W:2 * W],
                                ALU.mult)
        af_op("p4_0", "v")

        # more HAM fillers gated mid-chain
        warm(2, rhs=sp2b[:, 0:384])
        warm(2, rhs=c2b[:, 0:384])
        warm(1, rhs=p4_1[:, 0:384])
        warm(1, rhs=p8_1[:, 0:384])

        # ScalarE afs (post-sin shadow) + Exp table preload
        af_op("p1_0", "s")
        af_op("p4_1", "s")
        escr = wpool.tile([1, 1], F32, name="escr", tag="escr")
        nc.scalar.activation(escr[:], afs["p4_1"][0:1, 0:1], AF.Exp)

        # ---------------- main matmuls (readiness order) ----------------
        def harm(name):
            t, pt = afs[name], PT[name]
            for hc in range(2):
                for f in range(2):
                    main(t[:, hc * 256 + f * 128: hc * 256 + (f + 1) * 128],
                         pt[:, (1 - f) * W + hc * M + 128: (1 - f) * W + hc * M + M])

        harm("p1_0")
        harm("p4_1")
        harm("p8_1")
        for hc in range(2):   # corr8: u8 . sp8_k
            main(ukb[:, (4 + hc) * 128:(5 + hc) * 128],
                 p8_1[:, hc * M + 128: hc * M + M])
        harm("p2_0")
        for hc in range(2):   # corr4: u4 . sp4_k (only needs sp4_0, runs early)
            main(ukb[:, (2 + hc) * 128:(3 + hc) * 128],
                 p4_0[:, hc * M + 128: hc * M + M])
        harm("p4_0")
        assert mi[0] == nmain

        # ---------------- exp (no max subtraction, chunked) + AV ----------------
        E_t = wpool.tile([NQ, KPAD], BF, name="Et", tag="Et")
        ov_ps = pov.tile([NQ, 257], F32, name="ov", tag="ov")
        for jc in range(KC):
            nk0 = jc * 128
            nkw = min(128, KPAD - nk0)
            nc.scalar.activation(E_t[:, nk0:nk0 + nkw], sc_ps[:, nk0:nk0 + nkw],
                                 AF.Exp)
            ps = ptp.tile([128, 128], BF, name="tpe", tag="tp")
            nc.tensor.transpose(ps[0:nkw, :], E_t[:, nk0:nk0 + nkw], ident)
            et = wpool.tile([128, NQ], BF, name=f"et{jc % 2}", tag=f"et{jc % 2}")
            nc.vector.tensor_copy(et[0:nkw, :], ps[0:nkw, :])
            nc.tensor.matmul(ov_ps[:, 0:257], et[0:nkw, :],
                             vv[0:nkw, jc * 257:(jc + 1) * 257],
                             start=(jc == 0), stop=(jc == KC - 1))
        out_sb = wpool.tile([NQ, 257], F32, name="outsb", tag="outsb")
        nc.vector.tensor_copy(out_sb[:], ov_ps[:, 0:257])
        nc.scalar.dma_start(d_o[:], out_sb[:])

    nc.finalize()
    return nc


_CACHE = {}


def _plan(vl):
    """Key-shard plan (64-key granule): per-core KPAD and (batch, offset)."""
    units = [max(1, (int(v) + 63) // 64) for v in vl]
    L = 1
    while sum((c + L - 1) // L for c in units) > 8:
        L += 1
    KPAD = 64 * L
    assign = []
    for b in range(B):
        for i in range((units[b] + L - 1) // L):
            assign.append((b, i * KPAD))
    live = len(assign)
    while len(assign) < 8:
        assign.append((0, 0))
    return KPAD, assign, live


def _packT(x):
    """[rows, 256] f32 -> [128, 2*rows] bf16 with cols (dc, row)."""
    import ml_dtypes
    t = x.T.astype(ml_dtypes.bfloat16).reshape(2, 128, -1).transpose(1, 0, 2)
    return np.ascontiguousarray(t.reshape(128, -1))


def _in_maps(queries, keys, values, vl, Wq, Wk, wv_c, KPAD, assign):
    import ml_dtypes
    KC = (KPAD + 127) // 128
    wqT = _packT(Wq)      # [128, 512]
    wkT = _packT(Wk)
    wv1 = wv_c.reshape(H)
    uk = (C0 * (Wk.T @ wv1)).astype(np.float32)          # [256] d-space
    u4 = (-4.0 * BS[(0, 4)] * wv1).astype(np.float32)    # [256] h-space
    u8 = (-8.0 * BS[(1, 8)] * wv1).astype(np.float32)
    blocks = [uk[0:128], uk[128:256], u4[0:128], u4[128:256], u8[0:128], u8[128:256]]
    ukb = np.concatenate([np.broadcast_to(v[:, None], (128, 128)) for v in blocks],
                         axis=1)
    wv2 = wv1.reshape(2, 128).T            # [128(dd), 2(hc)]
    wvq = np.concatenate(
        [np.float32(AF_COEF[name]) * wv2 for name in AF_ORDER],
        axis=1).astype(np.float32)          # [128, 2*len(AF_ORDER)]
    wvq = np.ascontiguousarray(wvq)
    ident = np.eye(128, dtype=np.float32)
    qT_b = {}
    maps = []
    for (b, off) in assign:
        if b not in qT_b:
            qT_b[b] = _packT(queries[b])  # [128, 256]
        end = min(int(vl[b]), off + KPAD)
        nvalid = max(end - off, 0)
        kb = np.zeros((KPAD, H), dtype=np.float32)
        vb = np.zeros((KC * 128, 257), dtype=np.float32)
        if nvalid > 0:
            kb[:nvalid] = keys[b, off:end]
            vb[:nvalid, 0:256] = values[b, off:end]
            vb[:nvalid, 256] = 1.0
        vvt = vb.reshape(KC, 128, 257).transpose(1, 0, 2).reshape(128, KC * 257)
        in1a = np.concatenate([wkT.astype(np.float32),
                               _packT(kb).astype(np.float32)], axis=1)
        in1b = np.concatenate([qT_b[b].astype(np.float32),
                               wqT.astype(np.float32)], axis=1)
        in2 = np.concatenate([ident, ukb, vvt], axis=1)
        maps.append({
            "in1a": np.ascontiguousarray(in1a.astype(ml_dtypes.bfloat16)),
            "in1b": np.ascontiguousarray(in1b.astype(ml_dtypes.bfloat16)),
            "in2": np.ascontiguousarray(in2.astype(ml_dtypes.bfloat16)),
            "wvq": wvq,
        })
    return maps


def _combine(results, assign, live):
    ov = np.zeros((B, NQ, DV), dtype=np.float32)
    z = np.zeros((B, NQ, 1), dtype=np.float32)
    for c in range(live):
        b, _ = assign[c]
        o = results[c]["o"]
        ov[b] += o[:, 0:256]
        z[b] += o[:, 256:257]
    return ov / z


def kernel(queries, keys, values, valid_lens, Wq, Wk, wv):
    queries = np.ascontiguousarray(queries, dtype=np.float32)
    keys = np.ascontiguousarray(keys, dtype=np.float32)
    values = np.ascontiguousarray(values, dtype=np.float32)
    Wq = np.ascontiguousarray(Wq, dtype=np.float32)
    Wk = np.ascontiguousarray(Wk, dtype=np.float32)
    wv_c = np.ascontiguousarray(np.asarray(wv).reshape(H, 1), dtype=np.float32)
    vl = np.asarray(valid_lens).astype(np.int64).reshape(B)

    KPAD, assign, live = _plan(vl)
    if KPAD not in _CACHE:
        _CACHE[KPAD] = build_program(KPAD)
    nc = _CACHE[KPAD]

    maps = _in_maps(queries, keys, values, vl, Wq, Wk, wv_c, KPAD, assign)
    res = run_bass_kernel_spmd(nc, maps, list(range(8))).results
    return _combine(res, assign, live)


if __name__ == "__main__":
    d = np.load("/tmp/additive_attn_ref.npz")
    out = kernel(**{k: d[k] for k in
                    ["queries", "keys", "values", "valid_lens", "Wq", "Wk", "wv"]})
    ref = d["out"]
    print("rel err:", np.linalg.norm(out - ref) / np.linalg.norm(ref))
    print("max abs err:", np.abs(out - ref).max())


# revision 29
# speedup vs baseline: 1.1736x; 1.1736x over previous
"""Additive (Bahdanau) attention on 8 TRN2 NeuronCores.

scores[b,i,j] = sum_h wv_h * tanh(qp[b,i,h] + kp[b,j,h]),  qp = q@Wq.T, kp = k@Wk.T
masked softmax over j, then attn @ values.

Math: tanh(s) ~ c0*s + sum_n b_n sin(n w s) over two frequency ladders
L0=(w0,(1,2,4)) and L1=(w1,(4,8)); sin(w(q+k)) = sin(wq)cos(wk)+cos(wq)sin(wk)
turns the (B,NQ,NK,H) tanh contraction into TensorEngine matmuls over Fourier
features. ACT Sin is only accurate for |arg|<=3.15 so cosines come from
Sin(-w|x| + pi/2) and w0 is capped at 0.46; higher harmonics via double-angle
ladders (sp_n = sin(nwx)/n raw, interior cosines exactified, leaf harmonics
use ct_n = cos(n/2 wx)^2 with rank-1 beta corrections). Per-query-constant
score terms are dropped: softmax is row-invariant and the host divides by z.

Device-side structure:
- host pre-packs ALL inputs (transposed, bf16) in two large DMAs;
- q and k projections land in ONE psum tile laid out (hc, [q|k]) so every
  ladder op covers q-side and k-side of both h-chunks in one instruction;
- the ladder chain runs entirely on the DVE (gpsimd streaming poisons the
  shared SBUF port); the wv*coef folds run on ScalarE (Copy + AP scale) for
  the early harmonics and on DVE after the chain for the late ones;
- valid_lens mask is folded into zeroed value rows + an appended ones-column
  (z = sum(E) falls out of the AV matmul); exp runs without max-subtraction;
- dummy matmuls on a memset tile + feature-dependent fillers keep the PE's
  HAM clock gate warm so the main matmuls run at 2.4 GHz.

Sharding: keys are sharded across cores at 64-key granularity. Each core gets
(batch b, key-range) with a common per-core KPAD = 64*L chosen so the
ceil(vl_b/64) units of all batches bin-pack into 8 single-batch bins; every
core computes partial ov[b] = E@V and z[b] = sum(E) over its key range for ALL
128 queries of its batch, and the host combines: out = sum(ov) / sum(z).
"""
import sys
import numpy as np

try:
    import concourse.bass as bass
except ImportError:
    sys.path.insert(0, "/opt/trn_rl_repo")
    import concourse.bass as bass
import concourse.bacc as bacc
import concourse.mybir as mybir
from contextlib import ExitStack
from concourse.tile import TileContext
from concourse.bass_utils import run_bass_kernel_spmd

F32 = mybir.dt.float32
BF = mybir.dt.bfloat16
AF = mybir.ActivationFunctionType
ALU = mybir.AluOpType

B, NQ, NK, H, DV = 4, 128, 1024, 256, 256
PIHALF = float(np.pi / 2)

# tanh(x) ~ C0*x + sum b_(li,n) sin(n * w_li * x); weighted LSQ fit over N(0,sigma^2)
CFG = ((0.46, (1, 2, 4)), (0.34, (4, 8)))
SIGMA = 1.665


def _fit():
    xs = np.linspace(-6 * SIGMA, 6 * SIGMA, 8001)
    wts = np.exp(-xs ** 2 / (2 * SIGMA ** 2))
    cols = [xs] + [np.sin(n * w * xs) for (w, hs) in CFG for n in hs]
    A = np.stack(cols, 1)
    Wm = np.sqrt(wts)[:, None]
    coef, *_ = np.linalg.lstsq(A * Wm, np.tanh(xs) * Wm[:, 0], rcond=None)
    c0 = float(coef[0])
    bs = {}
    i = 1
    for li, (w, hs) in enumerate(CFG):
        for n in hs:
            bs[(li, n)] = float(coef[i]); i += 1
    return c0, bs


C0, BS = _fit()
W0, W1 = CFG[0][0], CFG[1][0]

# af coefficient per pair-tile: interior n -> n*b_n ; leaf n -> 2n*b_n.
# AF_ORDER is the feature-readiness order used for the af ops and wvq cols.
AF_ORDER = ("p1_0", "p4_1", "p8_1", "p2_0", "p4_0")
AF_COEF = {
    "p1_0": BS[(0, 1)],
    "p2_0": 2.0 * BS[(0, 2)],
    "p4_0": 8.0 * BS[(0, 4)],      # L0 leaf (n=4)
    "p4_1": 4.0 * BS[(1, 4)],      # L1 interior
    "p8_1": 16.0 * BS[(1, 8)],     # L1 leaf (n=8)
}


def build_program(KPAD):
    KC = (KPAD + 127) // 128
    M = 128 + KPAD                  # per-hc ladder width (q part | k part)
    S = ((M + 511) // 512) * 512    # bank-aligned hc stride in the prj psum tile
    W = 2 * M                       # full ladder width (both h-chunks)
    # input 1a (sync, first): wkT | kT — gates kprj -> the sin backbone;
    # input 1b: qT | wqT;  input 2: ident | ukb | vv
    N1A = 512 + 2 * KPAD
    N1B = 256 + 512
    N2 = 128 + 768 + KC * 257

    nc = bacc.Bacc("TRN2", target_bir_lowering=False, debug=False, num_devices=8)
    d_in1a = nc.declare_dram_parameter("in1a", [128, N1A], BF, isOutput=False)
    d_in1b = nc.declare_dram_parameter("in1b", [128, N1B], BF, isOutput=False)
    d_in2 = nc.declare_dram_parameter("in2", [128, N2], BF, isOutput=False)
    d_wvq = nc.declare_dram_parameter("wvq", [128, 2 * len(AF_ORDER)], F32,
                                      isOutput=False)
    d_o = nc.declare_dram_parameter("o", [NQ, 257], F32, isOutput=True)

    with TileContext(nc) as tc, ExitStack() as ex:
        cpool = ex.enter_context(tc.tile_pool(name="consts", bufs=1))
        fpool = ex.enter_context(tc.tile_pool(name="feat", bufs=1))
        wpool = ex.enter_context(tc.tile_pool(name="work", bufs=1))
        pprj = ex.enter_context(tc.tile_pool(name="pprj", bufs=1, space="PSUM"))
        psc = ex.enter_context(tc.tile_pool(name="psc", bufs=1, space="PSUM"))
        pov = ex.enter_context(tc.tile_pool(name="pov", bufs=1, space="PSUM"))
        ptp = ex.enter_context(
            tc.tile_pool(name="ptp", bufs=(1 if S > 512 else 2), space="PSUM"))
        pwm = ex.enter_context(tc.tile_pool(name="pwm", bufs=1, space="PSUM"))

        # ---------------- DMAs (one ring, ordered by need) ----------------
        in1a = cpool.tile([128, N1A], BF, name="in1a", tag="in1a")
        nc.sync.dma_start(in1a[:], d_in1a[:])
        in1b = cpool.tile([128, N1B], BF, name="in1b", tag="in1b")
        nc.sync.dma_start(in1b[:], d_in1b[:])
        in2 = cpool.tile([128, N2], BF, name="in2", tag="in2")
        nc.sync.dma_start(in2[:], d_in2[:])
        wvq = cpool.tile([128, 2 * len(AF_ORDER)], F32, name="wvq", tag="wvq")
        nc.sync.dma_start(wvq[:], d_wvq[:])
        wkT = in1a[:, 0:512]
        kT = in1a[:, 512:512 + 2 * KPAD]
        qT = in1b[:, 0:256]
        wqT = in1b[:, 256:768]
        ident = in2[:, 0:128]
        ukb = in2[:, 128:128 + 768]
        vv = in2[:, 896:896 + KC * 257]

        pihalf = cpool.tile([128, 1], F32, name="pihalf", tag="pihalf")
        nc.vector.memset(pihalf[:], PIHALF)
        # junk tile: lets PE warmup matmuls start before any DMA lands
        wj = cpool.tile([128, 384], BF, name="wj", tag="wj")
        nc.vector.memset(wj[:], 1.0)

        # PE warmup into a scratch psum bank: the initial N=384 burst spans
        # >3.4us so the HAM SHORT window actually fires and unthrottles the PE
        wps = pwm.tile([128, 512], F32, name="wps", tag="wps")
        wcnt = [0]

        def warm(k, rhs=None, n=384):
            for _ in range(k):
                nc.tensor.matmul(wps[:, 0:n], wj[:, 0:128],
                                 wj[:] if rhs is None else rhs,
                                 start=(wcnt[0] == 0), stop=False,
                                 skip_group_check=True)
                wcnt[0] += 1

        warm(7)

        # ---------------- projections into one psum tile ----------------
        # prj cols: hc*S + [0:128 q | 128:128+KPAD k]; k first (its DMA lands
        # first and it gates the sin backbone). k split at psum bank
        # boundaries when M > 512.
        prj = pprj.tile([128, 2 * S], F32, name="prj", tag="prj")
        kpieces = []
        a0 = 128
        while a0 < M:
            a1 = min(((a0 // 512) + 1) * 512, M)
            kpieces.append((a0, a1))
            a0 = a1
        for hc in range(2):
            for (a0, a1) in kpieces:
                for dc in range(2):
                    nc.tensor.matmul(prj[:, hc * S + a0: hc * S + a1],
                                     wkT[:, dc * 256 + hc * 128: dc * 256 + (hc + 1) * 128],
                                     kT[:, dc * KPAD + (a0 - 128): dc * KPAD + (a1 - 128)],
                                     start=(dc == 0), stop=(dc == 1))
        for hc in range(2):
            for dc in range(2):
                nc.tensor.matmul(prj[:, hc * S: hc * S + 128],
                                 wqT[:, dc * 256 + hc * 128: dc * 256 + (hc + 1) * 128],
                                 qT[:, dc * NQ:(dc + 1) * NQ],
                                 start=(dc == 0), stop=(dc == 1))

        prjV = prj[:].rearrange("p (a j) -> p a j", a=2)[:, :, 0:M]

        def v3(tile_slice):
            return tile_slice.rearrange("p (a j) -> p a j", a=2)

        # ---------------- feature tiles ----------------
        # pair tiles [128, 2*W]: cols = f*W + hc*M + [0:128 q | 128:M k]
        p1_0 = fpool.tile([128, 2 * W], BF, name="p1_0", tag="p1_0")
        p2_0 = fpool.tile([128, 2 * W], BF, name="p2_0", tag="p2_0")
        p4_0 = fpool.tile([128, 2 * W], BF, name="p4_0", tag="p4_0")
        p4_1 = fpool.tile([128, 2 * W], BF, name="p4_1", tag="p4_1")
        p8_1 = fpool.tile([128, 2 * W], BF, name="p8_1", tag="p8_1")
        PT = {"p1_0": p1_0, "p2_0": p2_0, "p4_0": p4_0, "p4_1": p4_1, "p8_1": p8_1}
        absx = fpool.tile([128, W], F32, name="absx", tag="absx")
        s1b = fpool.tile([128, W], BF, name="s1b", tag="s1b")
        c1b = fpool.tile([128, W], BF, name="c1b", tag="c1b")
        sp2b = fpool.tile([128, W], BF, name="sp2b", tag="sp2b")
        ct2b = fpool.tile([128, W], BF, name="ct2b", tag="ct2b")
        c2b = fpool.tile([128, W], BF, name="c2b", tag="c2b")
        ct4b = fpool.tile([128, W], BF, name="ct4b", tag="ct4b")
        ct2a = fpool.tile([128, W], BF, name="ct2a", tag="ct2a")

        # ---------------- beta-linear mains (only need kT) ----------------
        sc_ps = psc.tile([NQ, KPAD], F32, name="sc", tag="sc")
        nmain = 2 + 4 * 5 + 4
        mi = [0]

        def main(lhsT, rhs):
            nc.tensor.matmul(sc_ps[:, :], lhsT, rhs,
                             start=(mi[0] == 0), stop=(mi[0] == nmain - 1))
            mi[0] += 1

        for dc in range(2):
            main(ukb[:, dc * 128:(dc + 1) * 128], kT[:, dc * KPAD:(dc + 1) * KPAD])

        # ---------------- ladder heads (ACT); L1 first for chain latency ----------------
        nc.scalar.activation(v3(s1b[:]), prjV, AF.Sin, scale=float(W1))
        nc.scalar.activation(v3(absx[:]), prjV, AF.Abs)
        nc.scalar.activation(c1b[:], absx[:], AF.Sin, scale=float(-W1),
                             bias=pihalf[:, 0:1])
        nc.scalar.activation(v3(p1_0[:, 0:W]), prjV, AF.Sin, scale=float(W0))
        nc.scalar.activation(p1_0[:, W:2 * W], absx[:], AF.Sin, scale=float(-W0),
                             bias=pihalf[:, 0:1])

        # HAM fillers: junk matmuls gated on ladder outputs keep the PE's
        # activity window covered while it waits for the main matmul inputs
        warm(2, rhs=s1b[:, 0:384])
        warm(2, rhs=c1b[:, 0:384])

        # af tiles: wv*coef fold on the q-side features. Early harmonics run
        # on ScalarE (Copy with per-partition AP scale) in the post-sin
        # shadow; late harmonics on DVE woven into the chain.
        afs = {name: fpool.tile([128, 512], BF, name=f"af{name}", tag=f"af{name}")
               for name in AF_ORDER}

        def af_op(name, eng):
            ni = AF_ORDER.index(name)
            t = afs[name]
            src3 = PT[name][:].rearrange("p (f x) -> p f x", f=2)
            for hc in range(2):
                out_ap = t[:, hc * 256:(hc + 1) * 256].rearrange(
                    "p (f q) -> p f q", f=2)
                src_ap = src3[:, :, hc * M: hc * M + 128]
                sc1 = wvq[:, 2 * ni + hc: 2 * ni + hc + 1]
                if eng == "v":
                    nc.vector.tensor_scalar(out_ap, src_ap, sc1, None, ALU.mult)
                else:
                    nc.scalar.mul(out_ap, src_ap, sc1)

        # ---------------- chains (all DVE; af weaved at feature readiness) ----------------
        # L1: sp2b=s1b*c1b, ct2b=c1b^2, c2b=2ct2b-1, sp4_1=sp2b*c2b,
        #     ct4b=c2b^2, c4_1=2ct4b-1, sp8_1=sp4_1*c4_1, ct8_1=c4_1^2
        nc.vector.tensor_tensor(sp2b[:], s1b[:], c1b[:], ALU.mult)
        nc.vector.tensor_tensor(ct2b[:], c1b[:], c1b[:], ALU.mult)
        nc.vector.tensor_scalar(c2b[:], ct2b[:], 2.0, -1.0, ALU.mult, ALU.add)
        nc.vector.tensor_tensor(p4_1[:, 0:W], sp2b[:], c2b[:], ALU.mult)
        nc.vector.tensor_tensor(ct4b[:], c2b[:], c2b[:], ALU.mult)
        nc.vector.tensor_scalar(p4_1[:, W:2 * W], ct4b[:], 2.0, -1.0, ALU.mult, ALU.add)
        nc.vector.tensor_tensor(p8_1[:, 0:W], p4_1[:, 0:W], p4_1[:, W:2 * W], ALU.mult)
        nc.vector.tensor_tensor(p8_1[:, W:2 * W], p4_1[:, W:2 * W], p4_1[:, W:2 * W],
                                ALU.mult)
        af_op("p8_1", "v")
        # L0: sp2_0=s1_0*c1_0 -> p2_0 f0, ct2a=c1_0^2, c2_0=2ct2a-1 -> p2_0 f1,
        #     sp4_0=sp2_0*c2_0 -> p4_0 f0, ct4_0=c2_0^2 -> p4_0 f1
        nc.vector.tensor_tensor(p2_0[:, 0:W], p1_0[:, 0:W], p1_0[:, W:2 * W], ALU.mult)
        nc.vector.tensor_tensor(ct2a[:], p1_0[:, W:2 * W], p1_0[:, W:2 * W], ALU.mult)
        nc.vector.tensor_scalar(p2_0[:, W:2 * W], ct2a[:], 2.0, -1.0, ALU.mult, ALU.add)
        af_op("p2_0", "v")
        nc.vector.tensor_tensor(p4_0[:, 0:W], p2_0[:, 0:W], p2_0[:, W:2 * W], ALU.mult)
        nc.vector.tensor_tensor(p4_0[:, W:2 * W], p2_0[:, W:2 * W], p2_0[:, W:2 * W],
                                ALU.mult)
        af_op("p4_0", "v")

        # more HAM fillers gated mid-chain
        warm(2, rhs=sp2b[:, 0:384])
        warm(2, rhs=c2b[:, 0:384])
        warm(1, rhs=p4_1[:, 0:384])
        warm(1, rhs=p8_1[:, 0:384])

        # ScalarE afs (post-sin shadow) + Exp table preload
        af_op("p1_0", "s")
        af_op("p4_1", "s")
        escr = wpool.tile([1, 1], F32, name="escr", tag="escr")
        nc.scalar.activation(escr[:], afs["p4_1"][0:1, 0:1], AF.Exp)

        # ---------------- main matmuls (readiness order) ----------------
        def harm(name):
            t, pt = afs[name], PT[name]
            for hc in range(2):
                for f in range(2):
                    main(t[:, hc * 256 + f * 128: hc * 256 + (f + 1) * 128],
                         pt[:, (1 - f) * W + hc * M + 128: (1 - f) * W + hc * M + M])

        harm("p1_0")
        harm("p4_1")
        harm("p8_1")
        for hc in range(2):   # corr8: u8 . sp8_k
            main(ukb[:, (4 + hc) * 128:(5 + hc) * 128],
                 p8_1[:, hc * M + 128: hc * M + M])
        harm("p2_0")
        for hc in range(2):   # corr4: u4 . sp4_k (only needs sp4_0, runs early)
            main(ukb[:, (2 + hc) * 128:(3 + hc) * 128],
                 p4_0[:, hc * M + 128: hc * M + M])
        harm("p4_0")
        assert mi[0] == nmain

        # ---------------- exp (no max subtraction, chunked) + AV ----------------
        E_t = wpool.tile([NQ, KPAD], BF, name="Et", tag="Et")
        ov_ps = pov.tile([NQ, 257], F32, name="ov", tag="ov")
        for jc in range(KC):
            nk0 = jc * 128
            nkw = min(128, KPAD - nk0)
            nc.scalar.activation(E_t[:, nk0:nk0 + nkw], sc_ps[:, nk0:nk0 + nkw],
                                 AF.Exp)
            ps = ptp.tile([128, 128], BF, name="tpe", tag="tp")
            nc.tensor.transpose(ps[0:nkw, :], E_t[:, nk0:nk0 + nkw], ident)
            et = wpool.tile([128, NQ], BF, name=f"et{jc % 2}", tag=f"et{jc % 2}")
            nc.vector.tensor_copy(et[0:nkw, :], ps[0:nkw, :])
            nc.tensor.matmul(ov_ps[:, 0:257], et[0:nkw, :],
                             vv[0:nkw, jc * 257:(jc + 1) * 257],
                             start=(jc == 0), stop=(jc == KC - 1))
        out_sb = wpool.tile([NQ, 257], F32, name="outsb", tag="outsb")
        nc.vector.tensor_copy(out_sb[:], ov_ps[:, 0:257])
        nc.scalar.dma_start(d_o[:], out_sb[:])

    nc.finalize()
    return nc


_CACHE = {}


def _plan(vl):
    """Key-shard plan (64-key granule): per-core KPAD and (batch, offset)."""
    units = [max(1, (int(v) + 63) // 64) for v in vl]
    L = 1
    while sum((c + L - 1) // L for c in units) > 8:
        L += 1
    KPAD = 64 * L
    assign = []
    for b in range(B):
        for i in range((units[b] + L - 1) // L):
            assign.append((b, i * KPAD))
    live = len(assign)
    while len(assign) < 8:
        assign.append((0, 0))
    return KPAD, assign, live


def _packT(x):
    """[rows, 256] f32 -> [128, 2*rows] bf16 with cols (dc, row)."""
    import ml_dtypes
    t = x.T.astype(ml_dtypes.bfloat16).reshape(2, 128, -1).transpose(1, 0, 2)
    return np.ascontiguousarray(t.reshape(128, -1))


def _in_maps(queries, keys, values, vl, Wq, Wk, wv_c, KPAD, assign):
    import ml_dtypes
    KC = (KPAD + 127) // 128
    wqT = _packT(Wq)      # [128, 512]
    wkT = _packT(Wk)
    wv1 = wv_c.reshape(H)
    uk = (C0 * (Wk.T @ wv1)).astype(np.float32)          # [256] d-space
    u4 = (-4.0 * BS[(0, 4)] * wv1).astype(np.float32)    # [256] h-space
    u8 = (-8.0 * BS[(1, 8)] * wv1).astype(np.float32)
    blocks = [uk[0:128], uk[128:256], u4[0:128], u4[128:256], u8[0:128], u8[128:256]]
    ukb = np.concatenate([np.broadcast_to(v[:, None], (128, 128)) for v in blocks],
                         axis=1)
    wv2 = wv1.reshape(2, 128).T            # [128(dd), 2(hc)]
    wvq = np.concatenate(
        [np.float32(AF_COEF[name]) * wv2 for name in AF_ORDER],
        axis=1).astype(np.float32)          # [128, 2*len(AF_ORDER)]
    wvq = np.ascontiguousarray(wvq)
    ident = np.eye(128, dtype=np.float32)
    qT_b = {}
    maps = []
    for (b, off) in assign:
        if b not in qT_b:
            qT_b[b] = _packT(queries[b])  # [128, 256]
        end = min(int(vl[b]), off + KPAD)
        nvalid = max(end - off, 0)
        kb = np.zeros((KPAD, H), dtype=np.float32)
        vb = np.zeros((KC * 128, 257), dtype=np.float32)
        if nvalid > 0:
            kb[:nvalid] = keys[b, off:end]
            vb[:nvalid, 0:256] = values[b, off:end]
            vb[:nvalid, 256] = 1.0
        vvt = vb.reshape(KC, 128, 257).transpose(1, 0, 2).reshape(128, KC * 257)
        in1a = np.concatenate([wkT.astype(np.float32),
                               _packT(kb).astype(np.float32)], axis=1)
        in1b = np.concatenate([qT_b[b].astype(np.float32),
                               wqT.astype(np.float32)], axis=1)
        in2 = np.concatenate([ident, ukb, vvt], axis=1)
        maps.append({
            "in1a": np.ascontiguousarray(in1a.astype(ml_dtypes.bfloat16)),
            "in1b": np.ascontiguousarray(in1b.astype(ml_dtypes.bfloat16)),
            "in2": np.ascontiguousarray(in2.astype(ml_dtypes.bfloat16)),
            "wvq": wvq,
        })
    return maps


def _combine(results, assign, live):
    ov = np.zeros((B, NQ, DV), dtype=np.float32)
    z = np.zeros((B, NQ, 1), dtype=np.float32)
    for c in range(live):
        b, _ = assign[c]
        o = results[c]["o"]
        ov[b] += o[:, 0:256]
        z[b] += o[:, 256:257]
    return ov / z


def kernel(queries, keys, values, valid_lens, Wq, Wk, wv):
    queries = np.ascontiguousarray(queries, dtype=np.float32)
    keys = np.ascontiguousarray(keys, dtype=np.float32)
    values = np.ascontiguousarray(values, dtype=np.float32)
    Wq = np.ascontiguousarray(Wq, dtype=np.float32)
    Wk = np.ascontiguousarray(Wk, dtype=np.float32)
    wv_c = np.ascontiguousarray(np.asarray(wv).reshape(H, 1), dtype=np.float32)
    vl = np.asarray(valid_lens).astype(np.int64).reshape(B)

    KPAD, assign, live = _plan(vl)
    if KPAD not in _CACHE:
        _CACHE[KPAD] = build_program(KPAD)
    nc = _CACHE[KPAD]

    maps = _in_maps(queries, keys, values, vl, Wq, Wk, wv_c, KPAD, assign)
    res = run_bass_kernel_spmd(nc, maps, list(range(8))).results
    return _combine(res, assign, live)


if __name__ == "__main__":
    d = np.load("/tmp/additive_attn_ref.npz")
    out = kernel(**{k: d[k] for k in
                    ["queries", "keys", "values", "valid_lens", "Wq", "Wk", "wv"]})
    ref = d["out"]
    print("rel err:", np.linalg.norm(out - ref) / np.linalg.norm(ref))
    print("max abs err:", np.abs(out - ref).max())
